# revision 49
# baseline (speedup 1.0000x reference)
"""Trainium2 Bass kernel for causal GQA attention (B=2, S=2048, D=2048,
H=32, KVH=8, hd=64) with RoPE and output projection, running SPMD on 8
NeuronCores.

Sharding: tensor-parallel over heads (4-way) x data-parallel over batch
(2-way).  Core c (b = c//4, k = c%4) handles batch b and heads
8k..8k+8 (kv heads 2k, 2k+1).  Attention outputs are AllGathered within
each batch group of 4 cores; each core then computes a 512-wide
output-dim slice of the wo projection; the host assembles the full
output.

Structure (fused per-s-tile pipeline, bf16 with fp32 psum accumulation):
- Everything lives in transposed [feature, seq] form so head_dim sits on
  SBUF partitions; K is computed once and partition-duplicated with 4
  small DMAs (instead of duplicated-wk matmuls); V is PE-transposed into
  [kv, 64 ones | 64 v] stationaries so PV also produces the softmax
  denominators for free.
- Per tile st: Q-projection+RoPE, then attention (scores with K=64
  pairs on disjoint PE row groups running concurrently, exp on ACT,
  diagonal masks on DVE, PV), with per-half/per-quarter AllGather
  triggers as soon as each pair of heads drains.
- Attention is exp(ACT)-throughput-gated, so independent PE work is
  interleaved into it as filler: the NEXT tile's K/V projection (on the
  Z psum banks) and the wo matmuls of earlier tiles (gathered halves
  are consumed as they land; the last tile gathers in quarters and its
  wo accumulates quarter-by-quarter on the freed score banks so only 16
  matmuls depend on the final AllGather).
- Second-half attention epilogues are deferred past the next tile's
  RoPE so the DVE serves the critical path first; PSUM is statically
  partitioned X=4/Y=2/Z=2 banks; the PE clock is pre-warmed with
  throwaway transposes during the initial weight DMAs.
"""

import numpy as np

DIM = 2048
S = 2048
B = 2
H = 32
KVH = 8
HD = 64
P = 128
QT = 512        # q tile (free dim of score matmuls)
NQT = S // QT   # 4
NKV = S // P    # 16 kv tiles of 128
DK = DIM // P   # 16 contraction tiles
ROPE_BASE = 10000.0
N_CORES = 8

_CACHE = {}


def _build():
    import concourse.bacc as bacc
    import concourse.tile as tile
    import concourse.mybir as mybir
    from concourse.masks import make_identity

    F32 = mybir.dt.float32
    BF16 = mybir.dt.bfloat16
    FP8 = mybir.dt.float8e4
    DR = mybir.MatmulPerfMode.DoubleRow
    Exp = mybir.ActivationFunctionType.Exp

    nc = bacc.Bacc("TRN2", target_bir_lowering=False, debug=False,
                   num_devices=N_CORES)

    xT = nc.dram_tensor("xT", [DIM, S], BF16, kind="ExternalInput").ap()
    wqT = nc.dram_tensor("wqT", [DIM, 512], BF16, kind="ExternalInput").ap()
    wkT = nc.dram_tensor("wkT", [DIM, 128], BF16, kind="ExternalInput").ap()
    wvT = nc.dram_tensor("wvT", [DIM, 128], BF16, kind="ExternalInput").ap()
    woT = nc.dram_tensor("woT", [DIM, 512], BF16, kind="ExternalInput").ap()
    cosT = nc.dram_tensor("cosT", [P, S], BF16, kind="ExternalInput").ap()
    sinT = nc.dram_tensor("sinT", [P, S], BF16, kind="ExternalInput").ap()
    maskT = nc.dram_tensor("maskT", [P, 4, QT], BF16, kind="ExternalInput").ap()
    out_t = nc.dram_tensor("out_t", [512, S], F32, kind="ExternalOutput").ap()

    xT3 = xT.rearrange("(o p) s -> p o s", p=P)
    wqT3 = wqT.rearrange("(o p) f -> p o f", p=P)
    wkT3 = wkT.rearrange("(o p) f -> p o f", p=P)
    wvT3 = wvT.rearrange("(o p) f -> p o f", p=P)
    woT3 = woT.rearrange("(o p) f -> p o f", p=P)

    with tile.TileContext(nc) as tc:
        with (
            tc.tile_pool(name="pers", bufs=1) as pers,
            tc.tile_pool(name="ps", bufs=1, space="PSUM") as ps,
            tc.tile_pool(name="dram", bufs=1, space="DRAM") as dram,
        ):
            # ---- persistent SBUF tiles ----
            q_fin = [pers.tile([P, S], BF16, name=f"q_fin{m}") for m in range(4)]
            k_dup = [pers.tile([P, S], BF16, name=f"k_dup{g}") for g in range(2)]
            v1 = [pers.tile([P, NKV, P], BF16, name=f"v1_{g}") for g in range(2)]
            msk = pers.tile([P, 4, QT], BF16, name="msk")
            wq_sb = pers.tile([P, DK, 512], BF16, name="wq_sb")
            wkv_sb = pers.tile([P, DK, 256], BF16, name="wkv_sb")
            wo_sb = pers.tile([P, DK, 512], BF16, name="wo_sb")
            cos_sb = pers.tile([P, S], BF16, name="cos_sb")
            sin_sb = pers.tile([P, S], BF16, name="sin_sb")

            # per-tile AllGathers are split (heads 0-3 from attention
            # pairs 0,1 / heads 4-7 from pairs 2,3) so each AG triggers as
            # soon as its piece is ready and wo can start on early pieces
            # while later ones are on the wire.  The last tile is split
            # into quarters (one per attention pair) to minimise the tail.
            cc_in = [[dram.tile([256, QT], BF16, name=f"cc_in{t}_{hh}")
                      for hh in range(2)] for t in range(NQT - 1)]
            cc_out = [[dram.tile([4 * 256, QT], BF16, name=f"cc_out{t}_{hh}")
                       for hh in range(2)] for t in range(NQT - 1)]
            cc_in.append([dram.tile([128, QT], BF16, name=f"cc_in3_{qq}")
                          for qq in range(4)])
            cc_out.append([dram.tile([4 * 128, QT], BF16, name=f"cc_out3_{qq}")
                           for qq in range(4)])

            # PSUM layout (8 banks, 3 static tags):
            #   X: 2 bufs x [P,2,QT] (2 banks each) = 4 banks
            #      proj: qa/qb     attn: ss per (g2, hf)
            #   Y: 2 bufs x [P,QT] = 2 banks
            #      proj: k/v psums attn: pv per hf
            #   Z: 2 bufs x [P,QT] = 2 banks
            #      transposes pst + wo pw
            def Xt(name):
                return ps.tile([P, 2, QT], F32, tag="x2", bufs=2, name=name)

            def Yt(name):
                return ps.tile([P, QT], F32, tag="y1", bufs=2, name=name)

            def Zt(name):
                return ps.tile([P, QT], F32, tag="z1", bufs=2, name=name)

            # ---------------- preloads ----------------
            with tc.tile_pool(name="pa", bufs=1) as pa:
                # x for the whole kernel is streamed per s-tile into a
                # resident per-tile buffer (double buffered)
                def xst_alloc():
                    return pa.tile([P, DK, QT], BF16, tag="xst", bufs=2,
                                   name="xst")

                # order matters: the first proj matmul needs xst chunk 0 +
                # wq chunk 0 + wkv; cos/sin are needed right after proj(0)
                xst0 = xst_alloc()
                nc.sync.dma_start(xst0[:, 0:4, :], xT3[:, 0:4, 0:QT])
                nc.sync.dma_start(wq_sb[:, 0:4, :], wqT3[:, 0:4, :])
                nc.sync.dma_start(wkv_sb[:, :, 0:128], wkT3[:])
                nc.sync.dma_start(wkv_sb[:, :, 128:256], wvT3[:])
                nc.sync.dma_start(cos_sb[:], cosT[:])
                nc.sync.dma_start(sin_sb[:], sinT[:])
                nc.sync.dma_start(msk[:], maskT[:])
                for cch in range(1, 4):
                    nc.sync.dma_start(xst0[:, 4 * cch:4 * (cch + 1), :],
                                      xT3[:, 4 * cch:4 * (cch + 1), 0:QT])
                    nc.sync.dma_start(wq_sb[:, 4 * cch:4 * (cch + 1), :],
                                      wqT3[:, 4 * cch:4 * (cch + 1), :])
                nc.sync.dma_start(wo_sb[:], woT3[:])
                ident_f = pa.tile([P, P], F32)
                ident = pa.tile([P, P], BF16)
                make_identity(nc, ident_f[:])
                nc.vector.tensor_copy(ident[:], ident_f[:])

                # warm up the PE clock (HAM un-throttles after ~3.4us of
                # sustained activity) with throwaway transposes while the
                # first weight/x DMAs are still in flight
                for w in range(24):
                    wps = ps.tile([P, QT], BF16, tag="z1", bufs=2,
                                  name=f"warm{w}")
                    nc.tensor.transpose(wps[:, 0:P], ident[:], ident[:])

                # ones columns of the PV stationary operand
                ones1 = pa.tile([P, HD], F32)
                nc.vector.memset(ones1[:], 1.0)
                for g in range(2):
                    for j in range(NKV):
                        nc.vector.tensor_copy(v1[g][:, j, 0:HD], ones1[:])

                # ============ fused per-s-tile pipeline ============
                kv_state = {}

                def rope_chain(dst, src, ssl, raw_eng, mul_eng):
                    raw = pa.tile([P, QT], F32, tag="raw", bufs=4,
                                  name="raw")
                    raw_eng.copy(raw[:], src) if raw_eng is nc.scalar \
                        else raw_eng.tensor_copy(raw[:], src)
                    rot = pa.tile([P, QT], F32, tag="rot", bufs=3,
                                  name="rot")
                    for hh in range(2):
                        base = hh * HD
                        nc.sync.dma_start(rot[base:base + 32, :],
                                          raw[base + 32:base + 64, :])
                        nc.sync.dma_start(rot[base + 32:base + 64, :],
                                          raw[base:base + 32, :])
                    mul_eng.tensor_mul(rot[:], rot[:], sin_sb[:, ssl])
                    mul_eng.tensor_mul(raw[:], raw[:], cos_sb[:, ssl])
                    mul_eng.tensor_add(dst, raw[:], rot[:])

                def proj_kv_mm(st, xst, part):
                    """K/V projection matmuls for s-tile st (on the Z
                    banks so they can run inside the previous tile's
                    attention as PE filler).  part selects o-chunks."""
                    if part == 0:
                        kv_state[st] = (Zt(f"kk{st}"), Zt(f"vv{st}"))
                    kk, vv = kv_state[st]
                    os_ = range(0, 8) if part == 0 else range(8, DK)
                    for o in os_:
                        first = o == 0
                        last = o == DK - 1
                        nc.tensor.matmul(kk[:], wkv_sb[:, o, 0:128],
                                         xst[:, o, :], start=first, stop=last)
                        nc.tensor.matmul(vv[:], wkv_sb[:, o, 128:256],
                                         xst[:, o, :], start=first, stop=last)

                def proj_kv_fin(st):
                    """k RoPE + k-dup + v transpose for s-tile st."""
                    ssl = slice(st * QT, (st + 1) * QT)
                    kk, vv = kv_state.pop(st)
                    k_ro = pa.tile([P, QT], BF16, tag="kro", bufs=2,
                                   name="k_ro")
                    rope_chain(k_ro[:], kk[:], ssl, nc.vector, nc.gpsimd)
                    # duplicate each kv head's 64 rows onto both partition
                    # halves (replaces the old duplicated-wk matmuls)
                    for g in range(2):
                        ksrc = k_ro[g * HD:(g + 1) * HD, :]
                        for hf in range(2):
                            nc.sync.dma_start(
                                k_dup[g][hf * HD:(hf + 1) * HD, ssl], ksrc)
                    # V: drain + transpose into v1 blocks
                    v_sb = pa.tile([P, QT], BF16, tag="vsb", bufs=2,
                                   name="v_sb")
                    nc.scalar.copy(v_sb[:], vv[:])
                    for jj in range(4):
                        j = 4 * st + jj
                        pst = ps.tile([P, QT], BF16, tag="z1",
                                      bufs=2, name=f"pst{st}_{jj}")
                        nc.tensor.transpose(
                            pst[:, 0:P],
                            v_sb[:, jj * P:(jj + 1) * P], ident[:])
                        for g in range(2):
                            nc.vector.tensor_copy(
                                v1[g][:, j, HD:P],
                                pst[:, g * HD:(g + 1) * HD])

                def proj_q(st, xst, half=None):
                    """Q projections + RoPE for s-tile st.  half=0/1
                    projects only pairs 0,1 / 2,3 (used for the last tile
                    so its attention + exp stream starts earlier)."""
                    ssl = slice(st * QT, (st + 1) * QT)
                    mms = range(4) if half is None else \
                        range(2 * half, 2 * half + 2)
                    qt = {}
                    for m in mms:
                        if m % 2 == 0:
                            qt[m // 2] = Xt(f"q{st}_{m // 2}")
                    qps = {m: qt[m // 2][:, m % 2, :] for m in mms}
                    for o in range(DK):
                        first = o == 0
                        last = o == DK - 1
                        for m in mms:
                            nc.tensor.matmul(
                                qps[m], wq_sb[:, o, m * P:(m + 1) * P],
                                xst[:, o, :], start=first, stop=last)
                    for m in mms:
                        rope_chain(q_fin[m][:, ssl], qps[m], ssl,
                                   nc.vector if m % 2 == 0 else nc.scalar,
                                   nc.vector if m % 2 == 0 else nc.gpsimd)

                def attn_pair(st, m, defer_epi=False):
                    """Heads 2m, 2m+1: K=64 score matmuls on disjoint PE
                    row groups run concurrently.  With defer_epi, the
                    normalize chain after the psum-releasing ocp copy is
                    returned as a closure to be emitted later (so the next
                    tile's RoPE work gets the DVE first)."""
                    ngrp = 2 * (st + 1)
                    qsl = slice(st * QT, (st + 1) * QT)
                    g = m // 2
                    prs = [slice(0, HD), slice(HD, P)]
                    pspv = [Yt(f"pv_{st}_{m}_{hf}") for hf in range(2)]
                    e_pair = []
                    for g2 in range(ngrp):
                        pss = [Xt(f"ss_{st}_{m}_{g2}_{hf}") for hf in range(2)]
                        for i in range(2):
                            j = 2 * g2 + i
                            for hf in range(2):
                                nc.tensor.matmul(
                                    pss[hf][:, i, :],
                                    k_dup[g][prs[hf], j * P:(j + 1) * P],
                                    q_fin[m][prs[hf], qsl],
                                    start=True, stop=True)
                        e2 = []
                        for hf in range(2):
                            e = pa.tile([P, 2, QT], BF16, tag="exp", bufs=7,
                                        name="e2")
                            nc.scalar.activation(e[:], pss[hf][:], Exp,
                                                 scale=0.125)
                            cpair = g2 - 2 * st
                            if cpair >= 0:
                                nc.vector.tensor_mul(
                                    e[:], e[:],
                                    msk[:, 2 * cpair:2 * cpair + 2, :])
                            e2.append(e)
                        e_pair.append(e2)
                        if g2 >= 1:
                            gp = g2 - 1
                            for i in range(2):
                                j = 2 * gp + i
                                for hf in range(2):
                                    nc.tensor.matmul(
                                        pspv[hf][:], v1[g][:, j, :],
                                        e_pair[gp][hf][:, i, :],
                                        start=(j == 0), stop=False)
                    for i in range(2):
                        j = 2 * (ngrp - 1) + i
                        for hf in range(2):
                            nc.tensor.matmul(
                                pspv[hf][:], v1[g][:, j, :],
                                e_pair[ngrp - 1][hf][:, i, :],
                                start=(j == 0), stop=(j == 4 * st + 3))
                    epis = []
                    for hf in range(2):
                        h = 2 * m + hf
                        # full copy so the pv bank releases while the
                        # normalize chain continues from SBUF (deferred
                        # pairs copy on ACT to keep boundary DVE free)
                        ocp = pa.tile([P, QT], F32, tag="ocp", bufs=4,
                                      name="ocp")
                        nc.vector.tensor_copy(ocp[:], pspv[hf][:])

                        def epi(h=h, ocp=ocp):
                            recip = pa.tile([1, QT], F32, tag="recip",
                                            bufs=2, name="recip")
                            nc.vector.reciprocal_approx_fast(recip[:],
                                                             ocp[0:1, :])
                            # broadcast 1/L to partitions 64:128 via a
                            # DRAM bounce
                            rb = dram.tile([1, QT], F32, tag="rb", bufs=2,
                                           name="rb")
                            nc.sync.dma_start(rb[:], recip[:])
                            bcast = pa.tile([P, QT], F32, tag="bcast",
                                            bufs=2, name="bcast")
                            nc.sync.dma_start(bcast[HD:P, :],
                                              rb[:].to_broadcast((HD, QT)))
                            o_sb = pa.tile([P, QT], BF16, tag="osb",
                                           bufs=3, name="o_sb")
                            nc.vector.tensor_mul(o_sb[HD:P, :],
                                                 ocp[HD:P, :],
                                                 bcast[HD:P, :])
                            if st == NQT - 1:   # quarter pieces (per pair)
                                dst = cc_in[st][h // 2][
                                    (h % 2) * HD:(h % 2 + 1) * HD, :]
                            else:               # half pieces
                                dst = cc_in[st][h // 4][
                                    (h % 4) * HD:(h % 4 + 1) * HD, :]
                            nc.sync.dma_start(dst, o_sb[HD:P, :])

                        if defer_epi:
                            epis.append(epi)
                        else:
                            epi()
                    return epis

                def trig_ag(t, hh):
                    nc.gpsimd.collective_compute(
                        "AllGather",
                        mybir.AluOpType.bypass,
                        replica_groups=[[0, 1, 2, 3], [4, 5, 6, 7]],
                        ins=[cc_in[t][hh][:].opt()],
                        outs=[cc_out[t][hh][:].opt()],
                    )

                cct_tiles = {}

                def cct_load(t):
                    """Load both gathered halves of tile t into separate
                    SBUF tiles (separate so wo can consume half a while
                    half b's AllGather is still in flight).  Half hh's
                    chunk oo corresponds to global f-chunk
                    o = 4*(oo//2) + 2*hh + oo%2."""
                    ca = pa.tile([P, 8, QT], BF16, tag="cct", bufs=4,
                                 name=f"cct{t}a")
                    cb = pa.tile([P, 8, QT], BF16, tag="cct", bufs=4,
                                 name=f"cct{t}b")
                    cct_tiles[t] = (ca, cb)
                    for hh, ct in ((0, ca), (1, cb)):
                        cc3 = cc_out[t][hh][:].rearrange(
                            "(r o p) s -> p (r o) s", p=P, o=2)
                        nc.gpsimd.dma_start(ct[:], cc3[:])

                def cct3_load_q(qq):
                    """Per-quarter load for the last tile, emitted right
                    after its AG trigger so it lands ASAP.  Quarter qq's
                    chunk r corresponds to global f-chunk 4*r + qq."""
                    ct = pa.tile([P, 4, QT], BF16, tag="cct", bufs=4,
                                 name=f"cct3_{qq}")
                    cc3 = cc_out[NQT - 1][qq][:].rearrange(
                        "(r p) s -> p r s", p=P)
                    nc.gpsimd.dma_start(ct[:], cc3[:])
                    return ct

                def wo3(cq):
                    """wo for the last q-tile: all 4 output d-groups
                    accumulate quarter-by-quarter on the (now free) X
                    banks, so only the last quarter's 16 matmuls depend on
                    the final AllGather."""
                    t = NQT - 1
                    qsl = slice(t * QT, (t + 1) * QT)
                    pws = [Xt("wo3_a"), Xt("wo3_b")]
                    for qq in range(4):
                        for r in range(4):
                            for dd in range(4):
                                nc.tensor.matmul(
                                    pws[dd // 2][:, dd % 2, :],
                                    wo_sb[:, 4 * r + qq,
                                          dd * P:(dd + 1) * P],
                                    cq[qq][:, r, :],
                                    start=(qq == 0 and r == 0),
                                    stop=(qq == 3 and r == 3))
                    for dd in range(4):
                        ot = pa.tile([P, QT], F32, tag="ot", bufs=2,
                                     name="ot")
                        nc.vector.tensor_copy(ot[:], pws[dd // 2][:, dd % 2, :])
                        nc.sync.dma_start(out_t[dd * P:(dd + 1) * P, qsl],
                                          ot[:])

                def wo_d(t, d, last=False):
                    """One 128-wide output-dim group of wo for q-tile t."""
                    qsl = slice(t * QT, (t + 1) * QT)
                    ca, cb = cct_tiles[t]
                    pw = Zt(f"pw{t}_{d}")
                    for hh, ct in ((0, ca), (1, cb)):
                        for oo in range(8):
                            o = 4 * (oo // 2) + 2 * hh + oo % 2
                            nc.tensor.matmul(
                                pw[:], wo_sb[:, o, d * P:(d + 1) * P],
                                ct[:, oo, :],
                                start=(hh == 0 and oo == 0),
                                stop=(hh == 1 and oo == 7))
                    ot = pa.tile([P, QT], F32, tag="ot", bufs=2, name="ot")
                    nc.vector.tensor_copy(ot[:], pw[:])
                    nc.sync.dma_start(out_t[d * P:(d + 1) * P, qsl], ot[:])
                    if last:
                        cct_tiles.pop(t)

                # ---------------- main loop ----------------
                # wo(t) is interleaved into the attention of LATER s-tiles
                # (wo0 into attn2, wo1+wo2 into attn3) so a late AllGather
                # can never head-of-line-block independent PE work.  The
                # K/V projection of tile st+1 runs as PE filler inside
                # attention(st) (on the Z banks), so the ACT/exp-gated
                # attention keeps the PE busy and tile boundaries only
                # wait on the Q projection.  Second-half epilogues of
                # tiles 0-2 are deferred past the next tile's projection
                # so RoPE gets the DVE first.
                xsts = [xst0]
                cct3_tiles = []
                proj_kv_mm(0, xst0, 0)
                proj_kv_mm(0, xst0, 1)
                proj_kv_fin(0)
                deferred = []
                for st in range(NQT):
                    if st < NQT - 1:
                        nxt = xst_alloc()
                        nsl = slice((st + 1) * QT, (st + 2) * QT)
                        for cch in range(4):
                            nc.sync.dma_start(
                                nxt[:, 4 * cch:4 * (cch + 1), :],
                                xT3[:, 4 * cch:4 * (cch + 1), nsl])
                        xsts.append(nxt)
                    proj_q(st, xsts[st])
                    if deferred:
                        for epi in deferred:
                            epi()
                        deferred = []
                        trig_ag(st - 1, 1)
                    if st == 2:
                        cct_load(0)        # gpsimd queue: waits AG(0b)
                    elif st == 3:
                        cct_load(1)
                    for m in range(4):
                        epis = attn_pair(st, m,
                                         defer_epi=(m >= 2 and st < NQT - 1))
                        deferred.extend(epis)
                        if st == 3:
                            trig_ag(st, m)      # quarter AGs, one per pair
                            if m == 0:
                                cct_load(2)
                        elif m == 1:
                            trig_ag(st, 0)
                        if st < 2 and m in (0, 1):
                            proj_kv_mm(st + 1, xsts[st + 1], m)
                            if m == 1:
                                proj_kv_fin(st + 1)
                        if st == 2:
                            wo_d(0, m, last=(m == 3))
                            if m == 3:
                                proj_kv_mm(3, xsts[3], 0)
                                proj_kv_mm(3, xsts[3], 1)
                                proj_kv_fin(3)
                        elif st == 3:
                            wo_d(1 + m // 2, 2 * (m % 2), last=False)
                            wo_d(1 + m // 2, 2 * (m % 2) + 1,
                                 last=(m % 2 == 1))
                            if m % 2 == 1:
                                # the freed cct bufs let the next two
                                # quarter loads fire as soon as their AGs
                                # land (instead of all after the loop)
                                cct3_tiles.append(cct3_load_q(m - 1))
                                cct3_tiles.append(cct3_load_q(m))
                wo3(cct3_tiles)

    nc.compile()
    return nc


def _prep_inputs(x, position_ids, wq, wk, wv, wo):
    import ml_dtypes

    BF = ml_dtypes.bfloat16
    x = np.asarray(x, dtype=np.float32)
    pos = np.asarray(position_ids).reshape(-1).astype(np.int64)
    wqTf = np.asarray(wq, dtype=np.float32).T
    wkTf = np.asarray(wk, dtype=np.float32).T
    wvTf = np.asarray(wv, dtype=np.float32).T
    woTf = np.asarray(wo, dtype=np.float32).T

    inv = 1.0 / (ROPE_BASE ** (np.arange(0, HD, 2, dtype=np.float32) / HD))
    freqs = np.outer(pos.astype(np.float32), inv)  # [S, 32]
    pidx = np.arange(P) % 32
    sign = np.where((np.arange(P) % HD) < 32, -1.0, 1.0).astype(np.float32)
    cosT = np.ascontiguousarray(np.cos(freqs)[:, pidx].T).astype(BF)  # [P, S]
    sinT = np.ascontiguousarray(
        np.sin(freqs)[:, pidx].T * sign[:, None]).astype(BF)

    pg = np.arange(P)[:, None, None]
    cg = np.arange(4)[None, :, None]
    fg = np.arange(QT)[None, None, :]
    maskT = ((fg - pg - 128 * cg) >= 0).astype(BF)

    xT = [np.ascontiguousarray(x[b].T).astype(BF) for b in range(B)]

    in_maps = []
    for c in range(N_CORES):
        b, k = c // 4, c % 4
        in_maps.append({
            "xT": xT[b],
            "wqT": np.ascontiguousarray(wqTf[:, 512 * k:512 * (k + 1)]).astype(BF),
            "wkT": np.ascontiguousarray(wkTf[:, 128 * k:128 * (k + 1)]).astype(BF),
            "wvT": np.ascontiguousarray(wvTf[:, 128 * k:128 * (k + 1)]).astype(BF),
            "woT": np.ascontiguousarray(woTf[:, 512 * k:512 * (k + 1)]).astype(BF),
            "cosT": cosT,
            "sinT": sinT,
            "maskT": maskT,
        })
    return in_maps


LAST_EXEC_NS = None


def kernel(x, position_ids, wq, wk, wv, wo, _trace=False):
    import time

    from concourse import bass_utils

    if "nc" not in _CACHE:
        _CACHE["nc"] = _build()
    nc = _CACHE["nc"]

    in_maps = _prep_inputs(x, position_ids, wq, wk, wv, wo)
    res = None
    for attempt in range(3):
        try:
            res = bass_utils.run_bass_kernel_spmd(
                nc, in_maps, core_ids=list(range(N_CORES)), trace=_trace)
            break
        except Exception:
            # transient device hiccups usually clear on retry
            if attempt == 2:
                raise
            time.sleep(20 * (attempt + 1))

    global LAST_EXEC_NS
    LAST_EXEC_NS = res.exec_time_ns

    out = np.empty((B, S, DIM), dtype=np.float32)
    for c in range(N_CORES):
        b, k = c // 4, c % 4
        out[b, :, 512 * k:512 * (k + 1)] = res.results[c]["out_t"].T
    return out


# revision 53
# speedup vs baseline: 1.0191x; 1.0191x over previous
"""Trainium2 Bass kernel for causal GQA attention (B=2, S=2048, D=2048,
H=32, KVH=8, hd=64) with RoPE and output projection, running SPMD on 8
NeuronCores.

Sharding: tensor-parallel over heads (4-way) x data-parallel over batch
(2-way).  Core c (b = c//4, k = c%4) handles batch b and heads
8k..8k+8 (kv heads 2k, 2k+1).  Attention outputs are AllGathered within
each batch group of 4 cores; each core then computes a 512-wide
output-dim slice of the wo projection; the host assembles the full
output.

Structure (fused per-s-tile pipeline, bf16 with fp32 psum accumulation):
- Everything lives in transposed [feature, seq] form so head_dim sits on
  SBUF partitions; K is computed once and partition-duplicated with 4
  small DMAs (instead of duplicated-wk matmuls); V is PE-transposed into
  [kv, 64 ones | 64 v] stationaries so PV also produces the softmax
  denominators for free.
- Per tile st: Q-projection+RoPE, then attention (scores with K=64
  pairs on disjoint PE row groups running concurrently, exp on ACT,
  diagonal masks on DVE, PV), with per-half/per-quarter AllGather
  triggers as soon as each pair of heads drains.
- Attention is exp(ACT)-throughput-gated, so independent PE work is
  interleaved into it as filler: the NEXT tile's K/V projection (on the
  Z psum banks) and the wo matmuls of earlier tiles (gathered halves
  are consumed as they land; the last tile gathers in quarters and its
  wo accumulates quarter-by-quarter on the freed score banks so only 16
  matmuls depend on the final AllGather).
- Second-half attention epilogues are deferred past the next tile's
  RoPE so the DVE serves the critical path first; PSUM is statically
  partitioned X=4/Y=2/Z=2 banks; the PE clock is pre-warmed with
  throwaway transposes during the initial weight DMAs.
"""

import numpy as np

DIM = 2048
S = 2048
B = 2
H = 32
KVH = 8
HD = 64
P = 128
QT = 512        # q tile (free dim of score matmuls)
NQT = S // QT   # 4
NKV = S // P    # 16 kv tiles of 128
DK = DIM // P   # 16 contraction tiles
ROPE_BASE = 10000.0
N_CORES = 8

_CACHE = {}


def _build():
    import concourse.bacc as bacc
    import concourse.tile as tile
    import concourse.mybir as mybir
    from concourse.masks import make_identity

    F32 = mybir.dt.float32
    BF16 = mybir.dt.bfloat16
    FP8 = mybir.dt.float8e4
    DR = mybir.MatmulPerfMode.DoubleRow
    Exp = mybir.ActivationFunctionType.Exp

    nc = bacc.Bacc("TRN2", target_bir_lowering=False, debug=False,
                   num_devices=N_CORES)

    xT = nc.dram_tensor("xT", [DIM, S], BF16, kind="ExternalInput").ap()
    wqT = nc.dram_tensor("wqT", [DIM, 512], BF16, kind="ExternalInput").ap()
    wkT = nc.dram_tensor("wkT", [DIM, 128], BF16, kind="ExternalInput").ap()
    wvT = nc.dram_tensor("wvT", [DIM, 128], BF16, kind="ExternalInput").ap()
    woT = nc.dram_tensor("woT", [DIM, 512], BF16, kind="ExternalInput").ap()
    cosT = nc.dram_tensor("cosT", [P, S], BF16, kind="ExternalInput").ap()
    sinT = nc.dram_tensor("sinT", [P, S], BF16, kind="ExternalInput").ap()
    maskT = nc.dram_tensor("maskT", [P, 4, QT], BF16, kind="ExternalInput").ap()
    out_t = nc.dram_tensor("out_t", [512, S], F32, kind="ExternalOutput").ap()

    xT3 = xT.rearrange("(o p) s -> p o s", p=P)
    wqT3 = wqT.rearrange("(o p) f -> p o f", p=P)
    wkT3 = wkT.rearrange("(o p) f -> p o f", p=P)
    wvT3 = wvT.rearrange("(o p) f -> p o f", p=P)
    woT3 = woT.rearrange("(o p) f -> p o f", p=P)

    with tile.TileContext(nc) as tc:
        with (
            tc.tile_pool(name="pers", bufs=1) as pers,
            tc.tile_pool(name="ps", bufs=1, space="PSUM") as ps,
            tc.tile_pool(name="dram", bufs=1, space="DRAM") as dram,
        ):
            # ---- persistent SBUF tiles ----
            q_fin = [pers.tile([P, S], BF16, name=f"q_fin{m}") for m in range(4)]
            k_dup = [pers.tile([P, S], BF16, name=f"k_dup{g}") for g in range(2)]
            v1 = [pers.tile([P, NKV, P], BF16, name=f"v1_{g}") for g in range(2)]
            msk = pers.tile([P, 4, QT], BF16, name="msk")
            wq_sb = pers.tile([P, DK, 512], BF16, name="wq_sb")
            wkv_sb = pers.tile([P, DK, 256], BF16, name="wkv_sb")
            wo_sb = pers.tile([P, DK, 512], BF16, name="wo_sb")
            cos_sb = pers.tile([P, S], BF16, name="cos_sb")
            sin_sb = pers.tile([P, S], BF16, name="sin_sb")

            # per-tile AllGathers are split (heads 0-3 from attention
            # pairs 0,1 / heads 4-7 from pairs 2,3) so each AG triggers as
            # soon as its piece is ready and wo can start on early pieces
            # while later ones are on the wire.  The last tile is split
            # into quarters (one per attention pair) to minimise the tail.
            cc_in = [[dram.tile([256, QT], BF16, name=f"cc_in{t}_{hh}")
                      for hh in range(2)] for t in range(NQT - 1)]
            cc_out = [[dram.tile([4 * 256, QT], BF16, name=f"cc_out{t}_{hh}")
                       for hh in range(2)] for t in range(NQT - 1)]
            cc_in.append([dram.tile([128, QT], BF16, name=f"cc_in3_{qq}")
                          for qq in range(4)])
            cc_out.append([dram.tile([4 * 128, QT], BF16, name=f"cc_out3_{qq}")
                           for qq in range(4)])

            # PSUM layout (8 banks, 3 static tags):
            #   X: 2 bufs x [P,2,QT] (2 banks each) = 4 banks
            #      proj: qa/qb     attn: ss per (g2, hf)
            #   Y: 2 bufs x [P,QT] = 2 banks
            #      proj: k/v psums attn: pv per hf
            #   Z: 2 bufs x [P,QT] = 2 banks
            #      transposes pst + wo pw
            def Xt(name):
                return ps.tile([P, 2, QT], F32, tag="x2", bufs=2, name=name)

            def Yt(name):
                return ps.tile([P, QT], F32, tag="y1", bufs=2, name=name)

            def Zt(name):
                return ps.tile([P, QT], F32, tag="z1", bufs=2, name=name)

            # ---------------- preloads ----------------
            with tc.tile_pool(name="pa", bufs=1) as pa:
                # x for the whole kernel is streamed per s-tile into a
                # resident per-tile buffer (double buffered)
                def xst_alloc():
                    return pa.tile([P, DK, QT], BF16, tag="xst", bufs=2,
                                   name="xst")

                # order matters: the K/V projection of tile 0 runs first
                # and needs wkv + xst chunks 0,1; then proj_q needs wq.
                xst0 = xst_alloc()
                nc.sync.dma_start(xst0[:, 0:4, :], xT3[:, 0:4, 0:QT])
                nc.sync.dma_start(wkv_sb[:, :, 0:128], wkT3[:])
                nc.sync.dma_start(wkv_sb[:, :, 128:256], wvT3[:])
                nc.sync.dma_start(xst0[:, 4:8, :], xT3[:, 4:8, 0:QT])
                nc.sync.dma_start(wq_sb[:, 0:4, :], wqT3[:, 0:4, :])
                nc.sync.dma_start(cos_sb[:], cosT[:])
                nc.sync.dma_start(sin_sb[:], sinT[:])
                for cch in range(2, 4):
                    nc.sync.dma_start(xst0[:, 4 * cch:4 * (cch + 1), :],
                                      xT3[:, 4 * cch:4 * (cch + 1), 0:QT])
                for cch in range(1, 4):
                    nc.sync.dma_start(wq_sb[:, 4 * cch:4 * (cch + 1), :],
                                      wqT3[:, 4 * cch:4 * (cch + 1), :])
                nc.sync.dma_start(msk[:], maskT[:])
                nc.sync.dma_start(wo_sb[:], woT3[:])
                ident_f = pa.tile([P, P], F32)
                ident = pa.tile([P, P], BF16)
                make_identity(nc, ident_f[:])
                nc.vector.tensor_copy(ident[:], ident_f[:])

                # warm up the PE clock (HAM un-throttles after ~3.4us of
                # sustained activity) with throwaway transposes while the
                # first weight/x DMAs are still in flight
                for w in range(24):
                    wps = ps.tile([P, QT], BF16, tag="z1", bufs=2,
                                  name=f"warm{w}")
                    nc.tensor.transpose(wps[:, 0:P], ident[:], ident[:])

                # ones columns of the PV stationary operand
                ones1 = pa.tile([P, HD], F32)
                nc.vector.memset(ones1[:], 1.0)
                for g in range(2):
                    for j in range(NKV):
                        nc.vector.tensor_copy(v1[g][:, j, 0:HD], ones1[:])

                # ============ fused per-s-tile pipeline ============
                kv_state = {}

                def rope_chain(dst, src, ssl, raw_eng, mul_eng):
                    raw = pa.tile([P, QT], F32, tag="raw", bufs=4,
                                  name="raw")
                    raw_eng.copy(raw[:], src) if raw_eng is nc.scalar \
                        else raw_eng.tensor_copy(raw[:], src)
                    rot = pa.tile([P, QT], F32, tag="rot", bufs=3,
                                  name="rot")
                    for hh in range(2):
                        base = hh * HD
                        nc.sync.dma_start(rot[base:base + 32, :],
                                          raw[base + 32:base + 64, :])
                        nc.sync.dma_start(rot[base + 32:base + 64, :],
                                          raw[base:base + 32, :])
                    mul_eng.tensor_mul(rot[:], rot[:], sin_sb[:, ssl])
                    mul_eng.tensor_mul(raw[:], raw[:], cos_sb[:, ssl])
                    mul_eng.tensor_add(dst, raw[:], rot[:])

                def proj_kv_mm(st, xst, part, nparts=2):
                    """K/V projection matmuls for s-tile st (on the Z
                    banks so they can run inside the previous tile's
                    attention as PE filler).  part selects o-chunks."""
                    if part == 0:
                        kv_state[st] = (Zt(f"kk{st}"), Zt(f"vv{st}"))
                    kk, vv = kv_state[st]
                    opp = DK // nparts
                    os_ = range(part * opp, (part + 1) * opp)
                    for o in os_:
                        first = o == 0
                        last = o == DK - 1
                        nc.tensor.matmul(kk[:], wkv_sb[:, o, 0:128],
                                         xst[:, o, :], start=first, stop=last)
                        nc.tensor.matmul(vv[:], wkv_sb[:, o, 128:256],
                                         xst[:, o, :], start=first, stop=last)

                def proj_kv_fin(st):
                    """k RoPE + k-dup + v transpose for s-tile st."""
                    ssl = slice(st * QT, (st + 1) * QT)
                    kk, vv = kv_state.pop(st)
                    k_ro = pa.tile([P, QT], BF16, tag="kro", bufs=2,
                                   name="k_ro")
                    rope_chain(k_ro[:], kk[:], ssl, nc.vector, nc.gpsimd)
                    # duplicate each kv head's 64 rows onto both partition
                    # halves (replaces the old duplicated-wk matmuls)
                    for g in range(2):
                        ksrc = k_ro[g * HD:(g + 1) * HD, :]
                        for hf in range(2):
                            nc.sync.dma_start(
                                k_dup[g][hf * HD:(hf + 1) * HD, ssl], ksrc)
                    # V: drain + transpose into v1 blocks
                    v_sb = pa.tile([P, QT], BF16, tag="vsb", bufs=2,
                                   name="v_sb")
                    nc.scalar.copy(v_sb[:], vv[:])
                    for jj in range(4):
                        j = 4 * st + jj
                        pst = ps.tile([P, QT], BF16, tag="z1",
                                      bufs=2, name=f"pst{st}_{jj}")
                        nc.tensor.transpose(
                            pst[:, 0:P],
                            v_sb[:, jj * P:(jj + 1) * P], ident[:])
                        for g in range(2):
                            nc.vector.tensor_copy(
                                v1[g][:, j, HD:P],
                                pst[:, g * HD:(g + 1) * HD])

                def proj_q(st, xst, half=None):
                    """Q projections + RoPE for s-tile st.  half=0/1
                    projects only pairs 0,1 / 2,3 (used for the last tile
                    so its attention + exp stream starts earlier)."""
                    ssl = slice(st * QT, (st + 1) * QT)
                    mms = range(4) if half is None else \
                        range(2 * half, 2 * half + 2)
                    qt = {}
                    for m in mms:
                        if m % 2 == 0:
                            qt[m // 2] = Xt(f"q{st}_{m // 2}")
                    qps = {m: qt[m // 2][:, m % 2, :] for m in mms}
                    for o in range(DK):
                        first = o == 0
                        last = o == DK - 1
                        for m in mms:
                            nc.tensor.matmul(
                                qps[m], wq_sb[:, o, m * P:(m + 1) * P],
                                xst[:, o, :], start=first, stop=last)
                    for m in mms:
                        rope_chain(q_fin[m][:, ssl], qps[m], ssl,
                                   nc.vector if m % 2 == 0 else nc.scalar,
                                   nc.vector if m % 2 == 0 else nc.gpsimd)

                def attn_pair(st, m, defer_epi=False):
                    """Heads 2m, 2m+1: K=64 score matmuls on disjoint PE
                    row groups run concurrently.  With defer_epi, the
                    normalize chain after the psum-releasing ocp copy is
                    returned as a closure to be emitted later (so the next
                    tile's RoPE work gets the DVE first)."""
                    ngrp = 2 * (st + 1)
                    qsl = slice(st * QT, (st + 1) * QT)
                    g = m // 2
                    prs = [slice(0, HD), slice(HD, P)]
                    pspv = [Yt(f"pv_{st}_{m}_{hf}") for hf in range(2)]
                    e_pair = []
                    for g2 in range(ngrp):
                        pss = [Xt(f"ss_{st}_{m}_{g2}_{hf}") for hf in range(2)]
                        for i in range(2):
                            j = 2 * g2 + i
                            for hf in range(2):
                                nc.tensor.matmul(
                                    pss[hf][:, i, :],
                                    k_dup[g][prs[hf], j * P:(j + 1) * P],
                                    q_fin[m][prs[hf], qsl],
                                    start=True, stop=True)
                        e2 = []
                        for hf in range(2):
                            e = pa.tile([P, 2, QT], BF16, tag="exp", bufs=7,
                                        name="e2")
                            nc.scalar.activation(e[:], pss[hf][:], Exp,
                                                 scale=0.125)
                            cpair = g2 - 2 * st
                            if cpair >= 0:
                                nc.vector.tensor_mul(
                                    e[:], e[:],
                                    msk[:, 2 * cpair:2 * cpair + 2, :])
                            e2.append(e)
                        e_pair.append(e2)
                        if g2 >= 1:
                            gp = g2 - 1
                            for i in range(2):
                                j = 2 * gp + i
                                for hf in range(2):
                                    nc.tensor.matmul(
                                        pspv[hf][:], v1[g][:, j, :],
                                        e_pair[gp][hf][:, i, :],
                                        start=(j == 0), stop=False)
                    for i in range(2):
                        j = 2 * (ngrp - 1) + i
                        for hf in range(2):
                            nc.tensor.matmul(
                                pspv[hf][:], v1[g][:, j, :],
                                e_pair[ngrp - 1][hf][:, i, :],
                                start=(j == 0), stop=(j == 4 * st + 3))
                    epis = []
                    for hf in range(2):
                        h = 2 * m + hf
                        # full copy so the pv bank releases while the
                        # normalize chain continues from SBUF (deferred
                        # pairs copy on ACT to keep boundary DVE free)
                        ocp = pa.tile([P, QT], F32, tag="ocp", bufs=4,
                                      name="ocp")
                        nc.vector.tensor_copy(ocp[:], pspv[hf][:])

                        def epi(h=h, ocp=ocp):
                            recip = pa.tile([1, QT], F32, tag="recip",
                                            bufs=2, name="recip")
                            nc.vector.reciprocal_approx_fast(recip[:],
                                                             ocp[0:1, :])
                            # broadcast 1/L to partitions 64:128 via a
                            # DRAM bounce
                            rb = dram.tile([1, QT], F32, tag="rb", bufs=2,
                                           name="rb")
                            nc.sync.dma_start(rb[:], recip[:])
                            bcast = pa.tile([P, QT], F32, tag="bcast",
                                            bufs=2, name="bcast")
                            nc.sync.dma_start(bcast[HD:P, :],
                                              rb[:].to_broadcast((HD, QT)))
                            o_sb = pa.tile([P, QT], BF16, tag="osb",
                                           bufs=3, name="o_sb")
                            nc.vector.tensor_mul(o_sb[HD:P, :],
                                                 ocp[HD:P, :],
                                                 bcast[HD:P, :])
                            if st == NQT - 1:   # quarter pieces (per pair)
                                dst = cc_in[st][h // 2][
                                    (h % 2) * HD:(h % 2 + 1) * HD, :]
                            else:               # half pieces
                                dst = cc_in[st][h // 4][
                                    (h % 4) * HD:(h % 4 + 1) * HD, :]
                            nc.sync.dma_start(dst, o_sb[HD:P, :])

                        if defer_epi:
                            epis.append(epi)
                        else:
                            epi()
                    return epis

                def trig_ag(t, hh):
                    nc.gpsimd.collective_compute(
                        "AllGather",
                        mybir.AluOpType.bypass,
                        replica_groups=[[0, 1, 2, 3], [4, 5, 6, 7]],
                        ins=[cc_in[t][hh][:].opt()],
                        outs=[cc_out[t][hh][:].opt()],
                    )

                cct_tiles = {}

                def cct_load(t):
                    """Load both gathered halves of tile t into separate
                    SBUF tiles (separate so wo can consume half a while
                    half b's AllGather is still in flight).  Half hh's
                    chunk oo corresponds to global f-chunk
                    o = 4*(oo//2) + 2*hh + oo%2."""
                    ca = pa.tile([P, 8, QT], BF16, tag="cct", bufs=4,
                                 name=f"cct{t}a")
                    cb = pa.tile([P, 8, QT], BF16, tag="cct", bufs=4,
                                 name=f"cct{t}b")
                    cct_tiles[t] = (ca, cb)
                    for hh, ct in ((0, ca), (1, cb)):
                        cc3 = cc_out[t][hh][:].rearrange(
                            "(r o p) s -> p (r o) s", p=P, o=2)
                        nc.gpsimd.dma_start(ct[:], cc3[:])

                def cct3_load_q(qq):
                    """Per-quarter load for the last tile, emitted right
                    after its AG trigger so it lands ASAP.  Quarter qq's
                    chunk r corresponds to global f-chunk 4*r + qq."""
                    ct = pa.tile([P, 4, QT], BF16, tag="cct", bufs=4,
                                 name=f"cct3_{qq}")
                    cc3 = cc_out[NQT - 1][qq][:].rearrange(
                        "(r p) s -> p r s", p=P)
                    nc.gpsimd.dma_start(ct[:], cc3[:])
                    return ct

                def wo3(cq):
                    """wo for the last q-tile: all 4 output d-groups
                    accumulate quarter-by-quarter on the (now free) X
                    banks, so only the last quarter's 16 matmuls depend on
                    the final AllGather."""
                    t = NQT - 1
                    qsl = slice(t * QT, (t + 1) * QT)
                    pws = [Xt("wo3_a"), Xt("wo3_b")]
                    for qq in range(3):
                        for r in range(4):
                            for dd in range(4):
                                nc.tensor.matmul(
                                    pws[dd // 2][:, dd % 2, :],
                                    wo_sb[:, 4 * r + qq,
                                          dd * P:(dd + 1) * P],
                                    cq[qq][:, r, :],
                                    start=(qq == 0 and r == 0),
                                    stop=False)
                    # last quarter: finish each d-group's chain and drain
                    # it immediately so the final drains pipeline with the
                    # remaining matmuls instead of all queuing at the end
                    for dd in range(4):
                        for r in range(4):
                            nc.tensor.matmul(
                                pws[dd // 2][:, dd % 2, :],
                                wo_sb[:, 4 * r + 3, dd * P:(dd + 1) * P],
                                cq[3][:, r, :],
                                start=False, stop=(r == 3))
                        ot = pa.tile([P, QT], F32, tag="ot", bufs=2,
                                     name="ot")
                        nc.vector.tensor_copy(ot[:], pws[dd // 2][:, dd % 2, :])
                        nc.sync.dma_start(out_t[dd * P:(dd + 1) * P, qsl],
                                          ot[:])

                def wo_d(t, d, last=False):
                    """One 128-wide output-dim group of wo for q-tile t."""
                    qsl = slice(t * QT, (t + 1) * QT)
                    ca, cb = cct_tiles[t]
                    pw = Zt(f"pw{t}_{d}")
                    for hh, ct in ((0, ca), (1, cb)):
                        for oo in range(8):
                            o = 4 * (oo // 2) + 2 * hh + oo % 2
                            nc.tensor.matmul(
                                pw[:], wo_sb[:, o, d * P:(d + 1) * P],
                                ct[:, oo, :],
                                start=(hh == 0 and oo == 0),
                                stop=(hh == 1 and oo == 7))
                    ot = pa.tile([P, QT], F32, tag="ot", bufs=2, name="ot")
                    nc.vector.tensor_copy(ot[:], pw[:])
                    nc.sync.dma_start(out_t[d * P:(d + 1) * P, qsl], ot[:])
                    if last:
                        cct_tiles.pop(t)

                # ---------------- main loop ----------------
                # wo(t) is interleaved into the attention of LATER s-tiles
                # (wo0 into attn2, wo1+wo2 into attn3) so a late AllGather
                # can never head-of-line-block independent PE work.  The
                # K/V projection of tile st+1 runs as PE filler inside
                # attention(st) (on the Z banks), so the ACT/exp-gated
                # attention keeps the PE busy and tile boundaries only
                # wait on the Q projection.  Second-half epilogues of
                # tiles 0-2 are deferred past the next tile's projection
                # so RoPE gets the DVE first.
                xsts = [xst0]
                cct3_tiles = []
                proj_kv_mm(0, xst0, 0)
                proj_kv_mm(0, xst0, 1)
                proj_kv_fin(0)
                deferred = []
                for st in range(NQT):
                    if st < NQT - 1:
                        nxt = xst_alloc()
                        nsl = slice((st + 1) * QT, (st + 2) * QT)
                        for cch in range(4):
                            nc.sync.dma_start(
                                nxt[:, 4 * cch:4 * (cch + 1), :],
                                xT3[:, 4 * cch:4 * (cch + 1), nsl])
                        xsts.append(nxt)
                    proj_q(st, xsts[st])
                    if deferred:
                        for epi in deferred:
                            epi()
                        deferred = []
                        trig_ag(st - 1, 1)
                    if st == 2:
                        cct_load(0)        # gpsimd queue: waits AG(0b)
                    elif st == 3:
                        cct_load(1)
                    for m in range(4):
                        epis = attn_pair(st, m,
                                         defer_epi=(m >= 2 and st < NQT - 1))
                        deferred.extend(epis)
                        if st == 3:
                            trig_ag(st, m)      # quarter AGs, one per pair
                            if m == 0:
                                cct_load(2)
                        elif m == 1:
                            trig_ag(st, 0)
                        if st < 2:
                            proj_kv_mm(st + 1, xsts[st + 1], m, nparts=4)
                            if m == 3:
                                proj_kv_fin(st + 1)
                        if st == 2:
                            wo_d(0, m, last=(m == 3))
                            if m == 3:
                                proj_kv_mm(3, xsts[3], 0)
                                proj_kv_mm(3, xsts[3], 1)
                                proj_kv_fin(3)
                        elif st == 3:
                            wo_d(1 + m // 2, 2 * (m % 2), last=False)
                            wo_d(1 + m // 2, 2 * (m % 2) + 1,
                                 last=(m % 2 == 1))
                            if m % 2 == 1:
                                # the freed cct bufs let the next two
                                # quarter loads fire as soon as their AGs
                                # land (instead of all after the loop)
                                cct3_tiles.append(cct3_load_q(m - 1))
                                cct3_tiles.append(cct3_load_q(m))
                wo3(cct3_tiles)

    nc.compile()
    return nc


def _prep_inputs(x, position_ids, wq, wk, wv, wo):
    import ml_dtypes

    BF = ml_dtypes.bfloat16
    x = np.asarray(x, dtype=np.float32)
    pos = np.asarray(position_ids).reshape(-1).astype(np.int64)
    wqTf = np.asarray(wq, dtype=np.float32).T
    wkTf = np.asarray(wk, dtype=np.float32).T
    wvTf = np.asarray(wv, dtype=np.float32).T
    woTf = np.asarray(wo, dtype=np.float32).T

    inv = 1.0 / (ROPE_BASE ** (np.arange(0, HD, 2, dtype=np.float32) / HD))
    freqs = np.outer(pos.astype(np.float32), inv)  # [S, 32]
    pidx = np.arange(P) % 32
    sign = np.where((np.arange(P) % HD) < 32, -1.0, 1.0).astype(np.float32)
    cosT = np.ascontiguousarray(np.cos(freqs)[:, pidx].T).astype(BF)  # [P, S]
    sinT = np.ascontiguousarray(
        np.sin(freqs)[:, pidx].T * sign[:, None]).astype(BF)

    pg = np.arange(P)[:, None, None]
    cg = np.arange(4)[None, :, None]
    fg = np.arange(QT)[None, None, :]
    maskT = ((fg - pg - 128 * cg) >= 0).astype(BF)

    xT = [np.ascontiguousarray(x[b].T).astype(BF) for b in range(B)]

    in_maps = []
    for c in range(N_CORES):
        b, k = c // 4, c % 4
        in_maps.append({
            "xT": xT[b],
            "wqT": np.ascontiguousarray(wqTf[:, 512 * k:512 * (k + 1)]).astype(BF),
            "wkT": np.ascontiguousarray(wkTf[:, 128 * k:128 * (k + 1)]).astype(BF),
            "wvT": np.ascontiguousarray(wvTf[:, 128 * k:128 * (k + 1)]).astype(BF),
            "woT": np.ascontiguousarray(woTf[:, 512 * k:512 * (k + 1)]).astype(BF),
            "cosT": cosT,
            "sinT": sinT,
            "maskT": maskT,
        })
    return in_maps


LAST_EXEC_NS = None


def kernel(x, position_ids, wq, wk, wv, wo, _trace=False):
    import time

    from concourse import bass_utils

    if "nc" not in _CACHE:
        _CACHE["nc"] = _build()
    nc = _CACHE["nc"]

    in_maps = _prep_inputs(x, position_ids, wq, wk, wv, wo)
    res = None
    for attempt in range(3):
        try:
            res = bass_utils.run_bass_kernel_spmd(
                nc, in_maps, core_ids=list(range(N_CORES)), trace=_trace)
            break
        except Exception:
            # transient device hiccups usually clear on retry
            if attempt == 2:
                raise
            time.sleep(20 * (attempt + 1))

    global LAST_EXEC_NS
    LAST_EXEC_NS = res.exec_time_ns

    out = np.empty((B, S, DIM), dtype=np.float32)
    for c in range(N_CORES):
        b, k = c // 4, c % 4
        out[b, :, 512 * k:512 * (k + 1)] = res.results[c]["out_t"].T
    return out
